# revision 5
# baseline (speedup 1.0000x reference)
"""Trainium2 Bass kernel for nn_APrioriLatentSpace (vq_codebook).

Data-parallel over batch across 8 NeuronCores; per-centroid segment sums /
counts and the kl partial are combined with one packed AllReduce.

Self-contained: hardcodes shapes, applies local walrus workarounds, and does
all sharding/unsharding on the host inside kernel().
"""

import os
import sys
import types

import numpy as np

# ---------------------------------------------------------------------------
# Environment shims (axon NTFF profile hook + artifact upload stub)
# ---------------------------------------------------------------------------
if "antenv.axon_hooks" not in sys.modules:
    _m = types.ModuleType("antenv.axon_hooks")
    _m._hook = None

    def _set_hook(h):
        _m._hook = h

    def _get_hook():
        return _m._hook

    _m.set_axon_ntff_profile_hook = _set_hook
    _m.get_axon_ntff_profile_hook = _get_hook
    sys.modules["antenv.axon_hooks"] = _m

import bass_rust
import concourse.bass as bass
import concourse.mybir as mybir
import concourse.tile as tile
import concourse.bass_utils as bass_utils
from concourse.masks import make_identity
from concourse.vector_clock import ScopedClock

bass_utils.upload_artifacts = lambda tmpdir: "local://skipped"
from concourse.bass_utils import run_bass_kernel_spmd

F32 = mybir.dt.float32
F32R = mybir.dt.float32r
AF = mybir.ActivationFunctionType
ALU = mybir.AluOpType

# ---------------------------------------------------------------------------
# Walrus workaround: this compiler build rejects >1 embedded sem wait per
# engine instruction. Hoist extra waits onto single-wait nops just before the
# instruction on the same engine.
# ---------------------------------------------------------------------------


def _patched_drain_and_barrier(self, tick_clock, wait_clock):
    drain_inst = self.nc.sync.drain()
    wait_clock.add_sem_waits(
        drain_inst.ins, ScopedClock({None: tick_clock.global_clock})
    )
    si = drain_inst.ins.sync_info
    if si is not None and len(si.on_wait) > 1:
        waits = list(si.on_wait)
        ups = list(si.on_update)
        drain_inst.ins.sync_info = bass_rust.SyncInfo(
            on_wait=[waits[0]], on_update=ups
        )
        for w in waits[1:]:
            nop = self.nc.sync.nop(nofuse=True)
            nop.ins.sync_info = bass_rust.SyncInfo(on_wait=[w], on_update=[])

    self.nc.all_engine_barrier()
    assert self.sems is not None
    popped = self.nc._tile_sem_poison_stack.pop()
    assert popped is self._sem_poison
    self.nc.clear_and_free_semaphores(list(self.sems.allocated().values()))
    self.nc.all_engine_barrier()


tile.TileContext._drain_and_barrier = _patched_drain_and_barrier


def _split_multi_waits(nc, max_waits=1):
    ctr = 0
    for f in nc.m.functions:
        for bb in f.blocks:
            new = []
            changed = False
            for inst in bb.instructions:
                si = inst.sync_info
                if si is not None and len(si.on_wait) > max_waits:
                    waits = list(si.on_wait)
                    keep, extra = waits[-max_waits:], waits[:-max_waits]
                    for w in extra:
                        ctr += 1
                        new.append(
                            mybir.InstNoOp(
                                name=f"waitsplit_{ctr}",
                                engine=inst.engine,
                                bass_nofuse=True,
                                sync_info=mybir.SyncInfo(on_wait=[w], on_update=[]),
                            )
                        )
                    inst.sync_info = mybir.SyncInfo(
                        on_wait=keep, on_update=list(si.on_update)
                    )
                    changed = True
                new.append(inst)
            if changed:
                bb.instructions = new
    return ctr


# ---------------------------------------------------------------------------
# Problem shapes (hardcoded per spec)
# ---------------------------------------------------------------------------
B, D, L, O, H, R = 16384, 4096, 512, 4096, 128, 100
N_CORES = 8
BC = B // N_CORES          # 2048 batch rows per core
NBT = 4                    # batch tiles per core (512 each)
NBT_W = 512                # batch tile width
NSUB = 4                   # 128-row sub-chunks per batch tile
KD = D // 128              # 32 contraction chunks over D
KL4 = L // 128             # 4 contraction chunks over L


def _build():
    nc = bass.Bass(num_devices=N_CORES)

    # ---- DRAM parameters (inputs) ----
    xT = nc.declare_dram_parameter("xT", [D, BC], F32R, isOutput=False)
    eps = nc.declare_dram_parameter("eps", [BC, L], F32, isOutput=False)
    W1 = nc.declare_dram_parameter("W1", [D, H], F32R, isOutput=False)
    W2mu = nc.declare_dram_parameter("W2mu", [H, L], F32R, isOutput=False)
    W2lv = nc.declare_dram_parameter("W2lv", [H, L], F32R, isOutput=False)
    P = nc.declare_dram_parameter("P", [L, L], F32R, isOutput=False)
    Wd1 = nc.declare_dram_parameter("Wd1", [L, H], F32, isOutput=False)
    Wd2 = nc.declare_dram_parameter("Wd2", [H, O], F32R, isOutput=False)
    CTm2 = nc.declare_dram_parameter("CTm2", [L, R], F32, isOutput=False)  # -2*C.T
    cn = nc.declare_dram_parameter("cn", [1, R], F32, isOutput=False)      # |c|^2
    C = nc.declare_dram_parameter("C", [R, L], F32, isOutput=False)
    b1c = nc.declare_dram_parameter("b1c", [H, 1], F32, isOutput=False)
    bd1c = nc.declare_dram_parameter("bd1c", [H, 1], F32, isOutput=False)
    cmu = nc.declare_dram_parameter("cmu", [1, L], F32, isOutput=False)    # b2[:L] @ P
    b2lv = nc.declare_dram_parameter("b2lv", [1, L], F32, isOutput=False)  # b2[L:]
    bd2 = nc.declare_dram_parameter("bd2", [1, O], F32, isOutput=False)
    vc = nc.declare_dram_parameter("vc", [1, R], F32, isOutput=False)      # visit_counts

    # ---- DRAM parameters (outputs) ----
    out = nc.declare_dram_parameter("out", [BC, O], F32, isOutput=True)
    kl_o = nc.declare_dram_parameter("kl", [1, 1], F32, isOutput=True)
    ncnt_o = nc.declare_dram_parameter("new_counts", [1, R], F32, isOutput=True)
    ncen_o = nc.declare_dram_parameter("new_centroids", [R, L], F32, isOutput=True)

    with tile.TileContext(nc) as tc:
        with (
            tc.tile_pool(name="consts", bufs=1) as consts,
            tc.tile_pool(name="xin", bufs=12) as xin,
            tc.tile_pool(name="sb", bufs=2) as sb,
            tc.tile_pool(name="mupool", bufs=6) as mupool,
            tc.tile_pool(name="zpool", bufs=6) as zpool,
            tc.tile_pool(name="ztpool", bufs=6) as ztpool,
            tc.tile_pool(name="opool", bufs=4) as opool,
            tc.tile_pool(name="ps", bufs=4, space="PSUM") as ps,
            tc.tile_pool(name="pseg", bufs=1, space="PSUM") as pseg,
            tc.tile_pool(name="pcnt", bufs=1, space="PSUM") as pcnt,
            tc.tile_pool(name="dram", bufs=1, space="DRAM") as dram,
        ):
            # ---- resident constants ----
            w1t = consts.tile([128, KD, H], F32R)
            nc.sync.dma_start(w1t[:], W1.rearrange("(c p) h -> p c h", p=128))
            w2mut = consts.tile([H, L], F32R)
            nc.sync.dma_start(w2mut[:], W2mu[:])
            w2lvt = consts.tile([H, L], F32R)
            nc.sync.dma_start(w2lvt[:], W2lv[:])
            pt = consts.tile([128, KL4, L], F32R)
            nc.sync.dma_start(pt[:], P.rearrange("(c p) l -> p c l", p=128))
            wd1t = consts.tile([128, KL4, H], F32)
            nc.sync.dma_start(wd1t[:], Wd1.rearrange("(c p) h -> p c h", p=128))
            wd2t = consts.tile([H, O], F32R)
            nc.sync.dma_start(wd2t[:], Wd2[:])
            ctm2t = consts.tile([128, KL4, R], F32)
            nc.sync.dma_start(ctm2t[:], CTm2.rearrange("(c p) r -> p c r", p=128))
            cnt_ = consts.tile([1, R], F32)
            nc.sync.dma_start(cnt_[:], cn[:])
            b1t = consts.tile([H, 1], F32)
            nc.sync.dma_start(b1t[:], b1c[:])
            bd1t = consts.tile([H, 1], F32)
            nc.sync.dma_start(bd1t[:], bd1c[:])
            cmut = consts.tile([1, L], F32)
            nc.sync.dma_start(cmut[:], cmu[:])
            b2lvt = consts.tile([1, L], F32)
            nc.sync.dma_start(b2lvt[:], b2lv[:])
            bd2t = consts.tile([1, O], F32)
            nc.sync.dma_start(bd2t[:], bd2[:])

            ones_col = consts.tile([128, 1], F32)
            nc.vector.memset(ones_col[:], 1.0)
            ones_row = consts.tile([1, 128], F32)
            nc.vector.memset(ones_row[:], 1.0)
            ident = consts.tile([128, 128], F32)
            make_identity(nc, ident[:])
            klstrip = consts.tile([128, 48], F32)

            # persistent PSUM accumulators
            p_seg = pseg.tile([R, L], F32)
            p_cnt = pcnt.tile([1, R], F32)

            sub = 0  # global 128-row sub-chunk index (0..15)
            for bt in range(NBT):
                b0 = bt * NBT_W
                # ---- enc1: h1T = relu(W1.T @ xT + b1)  [H, 512] ----
                ph1 = ps.tile([H, NBT_W], F32, tag="ps")
                for kc in range(KD):
                    xt = xin.tile([128, NBT_W], F32R, tag="x")
                    nc.sync.dma_start(
                        xt[:], xT[kc * 128 : (kc + 1) * 128, b0 : b0 + NBT_W]
                    )
                    nc.tensor.matmul(
                        ph1[:], w1t[:, kc, :], xt[:],
                        start=(kc == 0), stop=(kc == KD - 1),
                    )
                h1 = sb.tile([H, NBT_W], F32R, tag="h1")
                nc.scalar.activation(h1[:], ph1[:], AF.Relu, bias=b1t[:])

                # ---- enc2-mu: muT chunks [128, 512] x4 ----
                muT = []
                for c in range(KL4):
                    pm = ps.tile([128, NBT_W], F32, tag="ps")
                    nc.tensor.matmul(
                        pm[:], w2mut[:, c * 128 : (c + 1) * 128], h1[:],
                        start=True, stop=True,
                    )
                    mt = mupool.tile([128, NBT_W], F32R, tag="mu")
                    nc.scalar.activation(mt[:], pm[:], AF.Copy)
                    muT.append(mt)

                zs = []
                for m in range(NSUB):
                    ms = slice(m * 128, (m + 1) * 128)
                    # ---- enc2-logvar (normal layout) + b2lv ----
                    pl = ps.tile([128, L], F32, tag="ps")
                    nc.tensor.matmul(pl[:], h1[:, ms], w2lvt[:], start=True, stop=False)
                    nc.tensor.matmul(pl[:], ones_row[:], b2lvt[:], start=False, stop=True)
                    lv = sb.tile([128, L], F32, tag="lv")
                    nc.scalar.activation(
                        lv[:], pl[:], AF.Copy,
                        accum_out=klstrip[:, sub : sub + 1],
                    )
                    sg = sb.tile([128, L], F32, tag="sg")
                    nc.scalar.activation(sg[:], lv[:], AF.Exp, scale=0.5)
                    sc1 = sb.tile([128, L], F32, tag="scr")
                    nc.scalar.activation(
                        sc1[:], lv[:], AF.Exp,
                        accum_out=klstrip[:, 16 + sub : 17 + sub],
                    )

                    # ---- mu_s (normal layout) + cmu ----
                    pmu = ps.tile([128, L], F32, tag="ps")
                    for kc in range(KL4):
                        nc.tensor.matmul(
                            pmu[:], muT[kc][:, ms], pt[:, kc, :],
                            start=(kc == 0), stop=False,
                        )
                    nc.tensor.matmul(pmu[:], ones_row[:], cmut[:], start=False, stop=True)
                    sc2 = sb.tile([128, L], F32, tag="scr")
                    nc.scalar.activation(
                        sc2[:], pmu[:], AF.Square,
                        accum_out=klstrip[:, 32 + sub : 33 + sub],
                    )

                    # ---- z = mu_s + eps * sigma ----
                    ep = sb.tile([128, L], F32, tag="ep")
                    nc.sync.dma_start(ep[:], eps[b0 + m * 128 : b0 + (m + 1) * 128, :])
                    es = sb.tile([128, L], F32, tag="es")
                    nc.vector.tensor_tensor(es[:], ep[:], sg[:], ALU.mult)
                    z = zpool.tile([128, L], F32, tag="z")
                    nc.vector.tensor_tensor(z[:], pmu[:], es[:], ALU.add)
                    zs.append(z)
                    sub += 1

                # ---- zT: transpose z tiles -> [128(Lc), 512b] x4 ----
                zT = []
                for c in range(KL4):
                    pz = ps.tile([128, NBT_W], F32, tag="ps")
                    for m in range(NSUB):
                        nc.tensor.transpose(
                            pz[:, m * 128 : (m + 1) * 128],
                            zs[m][:, c * 128 : (c + 1) * 128],
                            ident[:],
                        )
                    zt = ztpool.tile([128, NBT_W], F32, tag="zt")
                    nc.scalar.activation(zt[:], pz[:], AF.Copy)
                    zT.append(zt)

                # ---- dec1: h2T = relu(Wd1.T @ zT + bd1)  [H, 512] ----
                ph2 = ps.tile([H, NBT_W], F32, tag="ps")
                for c in range(KL4):
                    nc.tensor.matmul(
                        ph2[:], wd1t[:, c, :], zT[c][:],
                        start=(c == 0), stop=(c == KL4 - 1),
                    )
                h2 = sb.tile([H, NBT_W], F32R, tag="h2")
                nc.scalar.activation(h2[:], ph2[:], AF.Relu, bias=bd1t[:])

                for m in range(NSUB):
                    ms = slice(m * 128, (m + 1) * 128)
                    gsub = bt * NSUB + m
                    # ---- scores S = -2 z C^T + |c|^2 (f32) ----
                    psc = ps.tile([128, R], F32, tag="ps")
                    for c in range(KL4):
                        nc.tensor.matmul(
                            psc[:], zT[c][:, ms], ctm2t[:, c, :],
                            start=(c == 0), stop=False,
                        )
                    nc.tensor.matmul(psc[:], ones_row[:], cnt_[:], start=False, stop=True)
                    mn = sb.tile([128, 1], F32, tag="mn")
                    nc.vector.tensor_reduce(mn[:], psc[:], mybir.AxisListType.X, ALU.min)
                    A = sb.tile([128, R], F32, tag="A")
                    nc.vector.tensor_scalar(A[:], psc[:], mn[:], None, ALU.is_equal)

                    # ---- segment sums + counts ----
                    nc.tensor.matmul(
                        p_seg[:], A[:], zs[m][:],
                        start=(gsub == 0), stop=(gsub == NBT * NSUB - 1),
                    )
                    nc.tensor.matmul(
                        p_cnt[:], ones_col[:], A[:],
                        start=(gsub == 0), stop=(gsub == NBT * NSUB - 1),
                    )

                    # ---- dec2: out rows [128, O] ----
                    for n in range(O // 512):
                        po = ps.tile([128, 512], F32, tag="ps")
                        nc.tensor.matmul(
                            po[:], h2[:, ms], wd2t[:, n * 512 : (n + 1) * 512],
                            start=True, stop=False,
                        )
                        nc.tensor.matmul(
                            po[:], ones_row[:], bd2t[:, n * 512 : (n + 1) * 512],
                            start=False, stop=True,
                        )
                        ot = opool.tile([128, 512], F32, tag="ot")
                        if n % 2 == 0:
                            nc.scalar.activation(ot[:], po[:], AF.Copy)
                        else:
                            nc.vector.tensor_copy(ot[:], po[:])
                        nc.sync.dma_start(
                            out[b0 + m * 128 : b0 + (m + 1) * 128, n * 512 : (n + 1) * 512],
                            ot[:],
                        )

            # ---- kl partial: sum(klstrip_lv) - sum(mu2) - sum(s2) over all ----
            kl_lv = sb.tile([128, 1], F32)
            nc.vector.tensor_reduce(kl_lv[:], klstrip[:, 0:16], mybir.AxisListType.X, ALU.add)
            kl_s2 = sb.tile([128, 1], F32)
            nc.vector.tensor_reduce(kl_s2[:], klstrip[:, 16:32], mybir.AxisListType.X, ALU.add)
            kl_m2 = sb.tile([128, 1], F32)
            nc.vector.tensor_reduce(kl_m2[:], klstrip[:, 32:48], mybir.AxisListType.X, ALU.add)
            kl_a = sb.tile([128, 1], F32)
            nc.vector.tensor_tensor(kl_a[:], kl_lv[:], kl_s2[:], ALU.subtract)
            kl_b = sb.tile([128, 1], F32)
            nc.vector.tensor_tensor(kl_b[:], kl_a[:], kl_m2[:], ALU.subtract)
            p_kl = ps.tile([1, 1], F32, tag="ps")
            nc.tensor.matmul(p_kl[:], kl_b[:], ones_col[:], start=True, stop=True)

            # ---- pack AllReduce bounce: rows 0..99 sums, row 100 counts, row 101 kl ----
            ar_in = dram.tile([R + 2, L], F32)
            ar_out = dram.tile([R + 2, L], F32)
            seg_sb = sb.tile([R, L], F32)
            nc.scalar.activation(seg_sb[:], p_seg[:], AF.Copy)
            nc.sync.dma_start(ar_in[0:R, :], seg_sb[:])
            cnt_sb = sb.tile([1, L], F32)
            nc.vector.memset(cnt_sb[:], 0.0)
            nc.vector.tensor_copy(cnt_sb[:, 0:R], p_cnt[:])
            nc.sync.dma_start(ar_in[R : R + 1, :], cnt_sb[:])
            kl_sb = sb.tile([1, L], F32)
            nc.vector.memset(kl_sb[:], 0.0)
            nc.vector.tensor_copy(kl_sb[:, 0:1], p_kl[:])
            nc.sync.dma_start(ar_in[R + 1 : R + 2, :], kl_sb[:])

            nc.gpsimd.collective_compute(
                "AllReduce",
                ALU.add,
                replica_groups=[list(range(N_CORES))],
                ins=[ar_in[:]],
                outs=[ar_out[:]],
            )

            # ---- post-AR (replicated on all cores) ----
            sums_t = sb.tile([R, L], F32)
            nc.sync.dma_start(sums_t[:], ar_out[0:R, :])
            cnt_row = sb.tile([1, R], F32)
            nc.sync.dma_start(cnt_row[:], ar_out[R : R + 1, 0:R])
            cnt_col = sb.tile([R, 1], F32)
            nc.sync.dma_start(
                cnt_col[:], ar_out[R : R + 1, 0:R].rearrange("a (r o) -> (a r) o", o=1)
            )
            klv = sb.tile([1, 1], F32)
            nc.sync.dma_start(klv[:], ar_out[R + 1 : R + 2, 0:1])

            # kl_out = -0.005 * (L*B + klsum) / B
            kl_fin = sb.tile([1, 1], F32)
            nc.scalar.activation(
                kl_fin[:], klv[:], AF.Copy,
                scale=float(-0.005 / B), bias=float(-0.005 * L),
            )
            nc.sync.dma_start(kl_o[:], kl_fin[:])

            # new_counts = visit_counts + counts
            vct = sb.tile([1, R], F32)
            nc.sync.dma_start(vct[:], vc[:])
            ncr = sb.tile([1, R], F32)
            nc.vector.tensor_tensor(ncr[:], vct[:], cnt_row[:], ALU.add)
            nc.sync.dma_start(ncnt_o[:], ncr[:])

            # centroid update
            mask = sb.tile([R, 1], F32)
            nc.vector.tensor_scalar(mask[:], cnt_col[:], 0.0, None, ALU.is_gt)
            mx = sb.tile([R, 1], F32)
            nc.vector.tensor_scalar(mx[:], cnt_col[:], 1.0, None, ALU.max)
            rec = sb.tile([R, 1], F32)
            nc.vector.reciprocal(rec[:], mx[:])
            c2a = sb.tile([R, 1], F32)
            nc.vector.tensor_tensor(c2a[:], rec[:], mask[:], ALU.mult)
            coef2 = sb.tile([R, 1], F32)
            nc.vector.tensor_scalar(coef2[:], c2a[:], 0.01, None, ALU.mult)
            coef1 = sb.tile([R, 1], F32)
            nc.vector.tensor_scalar(coef1[:], mask[:], -0.01, 1.0, ALU.mult, ALU.add)

            c_t = sb.tile([R, L], F32)
            nc.sync.dma_start(c_t[:], C[:])
            t1 = sb.tile([R, L], F32)
            nc.vector.tensor_scalar(t1[:], c_t[:], coef1[:], None, ALU.mult)
            t2 = sb.tile([R, L], F32)
            nc.vector.tensor_scalar(t2[:], sums_t[:], coef2[:], None, ALU.mult)
            newc = sb.tile([R, L], F32)
            nc.vector.tensor_tensor(newc[:], t1[:], t2[:], ALU.add)
            nc.sync.dma_start(ncen_o[:], newc[:])

    _split_multi_waits(nc)
    return nc


_NC_CACHE = None


def _get_nc():
    global _NC_CACHE
    if _NC_CACHE is None:
        _NC_CACHE = _build()
    return _NC_CACHE


def kernel(x, eps, W1, b1, W2, b2, prior_basis, Wd1, bd1, Wd2, bd2,
           visit_counts, visit_centroids):
    x = np.asarray(x, np.float32)
    eps_np = np.asarray(eps, np.float32)
    W1 = np.asarray(W1, np.float32)
    b1 = np.asarray(b1, np.float32)
    W2 = np.asarray(W2, np.float32)
    b2 = np.asarray(b2, np.float32)
    P = np.asarray(prior_basis, np.float32)
    Wd1 = np.asarray(Wd1, np.float32)
    bd1 = np.asarray(bd1, np.float32)
    Wd2 = np.asarray(Wd2, np.float32)
    bd2 = np.asarray(bd2, np.float32)
    vc = np.asarray(visit_counts, np.float32)
    C = np.asarray(visit_centroids, np.float32)

    cmu = (b2[:L] @ P).reshape(1, L).astype(np.float32)
    shared = {
        "W1": W1,
        "W2mu": np.ascontiguousarray(W2[:, :L]),
        "W2lv": np.ascontiguousarray(W2[:, L:]),
        "P": P,
        "Wd1": Wd1,
        "Wd2": Wd2,
        "CTm2": np.ascontiguousarray(-2.0 * C.T),
        "cn": (C * C).sum(axis=1).reshape(1, R).astype(np.float32),
        "C": C,
        "b1c": b1.reshape(H, 1),
        "bd1c": bd1.reshape(H, 1),
        "cmu": cmu,
        "b2lv": b2[L:].reshape(1, L),
        "bd2": bd2.reshape(1, O),
        "vc": vc.reshape(1, R),
    }
    in_maps = []
    for c in range(N_CORES):
        sl = slice(c * BC, (c + 1) * BC)
        m = dict(shared)
        m["xT"] = np.ascontiguousarray(x[sl].T)
        m["eps"] = np.ascontiguousarray(eps_np[sl])
        in_maps.append(m)

    nc = _get_nc()
    trace = bool(os.environ.get("BASS_KERNEL_TRACE"))
    if trace:
        from trn_agent_boot.trn_boot import _ntff_profile_via_ctypes
        import antenv.axon_hooks as ah

        if ah.get_axon_ntff_profile_hook() is None:
            ah.set_axon_ntff_profile_hook(
                _ntff_profile_via_ctypes("/opt/axon/libaxon_pjrt.so")
            )
    res = run_bass_kernel_spmd(nc, in_maps, list(range(N_CORES)), trace=trace)
    if trace:
        print(f"HW exec time: {res.exec_time_ns} ns")

    out = np.concatenate([res.results[c]["out"] for c in range(N_CORES)], axis=0)
    kl = np.float32(res.results[0]["kl"][0, 0])
    new_counts = res.results[0]["new_counts"].reshape(R)
    new_centroids = res.results[0]["new_centroids"]
    return out, kl, new_counts, new_centroids


# revision 10
# speedup vs baseline: 1.4183x; 1.4183x over previous
"""Trainium2 Bass kernel for nn_APrioriLatentSpace (vq_codebook).

Data-parallel over batch across 8 NeuronCores; per-centroid segment sums /
counts and the kl partial are combined with one packed AllReduce.

Self-contained: hardcodes shapes, applies local walrus workarounds, and does
all sharding/unsharding on the host inside kernel().
"""

import os
import sys
import types

import numpy as np

# ---------------------------------------------------------------------------
# Environment shims (axon NTFF profile hook + artifact upload stub)
# ---------------------------------------------------------------------------
if "antenv.axon_hooks" not in sys.modules:
    _m = types.ModuleType("antenv.axon_hooks")
    _m._hook = None

    def _set_hook(h):
        _m._hook = h

    def _get_hook():
        return _m._hook

    _m.set_axon_ntff_profile_hook = _set_hook
    _m.get_axon_ntff_profile_hook = _get_hook
    sys.modules["antenv.axon_hooks"] = _m

import bass_rust
import concourse.bass as bass
import concourse.mybir as mybir
import concourse.tile as tile
import concourse.bass_utils as bass_utils
from concourse.masks import make_identity
from concourse.vector_clock import ScopedClock

bass_utils.upload_artifacts = lambda tmpdir: "local://skipped"
from concourse.bass_utils import run_bass_kernel_spmd

F32 = mybir.dt.float32
F32R = mybir.dt.float32r
AF = mybir.ActivationFunctionType
ALU = mybir.AluOpType

# ---------------------------------------------------------------------------
# Walrus workaround: this compiler build rejects >1 embedded sem wait per
# engine instruction. Hoist extra waits onto single-wait nops just before the
# instruction on the same engine.
# ---------------------------------------------------------------------------


def _patched_drain_and_barrier(self, tick_clock, wait_clock):
    drain_inst = self.nc.sync.drain()
    wait_clock.add_sem_waits(
        drain_inst.ins, ScopedClock({None: tick_clock.global_clock})
    )
    si = drain_inst.ins.sync_info
    if si is not None and len(si.on_wait) > 1:
        waits = list(si.on_wait)
        ups = list(si.on_update)
        drain_inst.ins.sync_info = bass_rust.SyncInfo(
            on_wait=[waits[0]], on_update=ups
        )
        for w in waits[1:]:
            nop = self.nc.sync.nop(nofuse=True)
            nop.ins.sync_info = bass_rust.SyncInfo(on_wait=[w], on_update=[])

    self.nc.all_engine_barrier()
    assert self.sems is not None
    popped = self.nc._tile_sem_poison_stack.pop()
    assert popped is self._sem_poison
    self.nc.clear_and_free_semaphores(list(self.sems.allocated().values()))
    self.nc.all_engine_barrier()


tile.TileContext._drain_and_barrier = _patched_drain_and_barrier


def _split_multi_waits(nc, max_waits=1):
    ctr = 0
    for f in nc.m.functions:
        for bb in f.blocks:
            new = []
            changed = False
            for inst in bb.instructions:
                si = inst.sync_info
                if si is not None and len(si.on_wait) > max_waits:
                    waits = list(si.on_wait)
                    keep, extra = waits[-max_waits:], waits[:-max_waits]
                    for w in extra:
                        ctr += 1
                        new.append(
                            mybir.InstNoOp(
                                name=f"waitsplit_{ctr}",
                                engine=inst.engine,
                                bass_nofuse=True,
                                sync_info=mybir.SyncInfo(on_wait=[w], on_update=[]),
                            )
                        )
                    inst.sync_info = mybir.SyncInfo(
                        on_wait=keep, on_update=list(si.on_update)
                    )
                    changed = True
                new.append(inst)
            if changed:
                bb.instructions = new
    return ctr


# ---------------------------------------------------------------------------
# Problem shapes (hardcoded per spec)
# ---------------------------------------------------------------------------
B, D, L, O, H, R = 16384, 4096, 512, 4096, 128, 100
N_CORES = 8
BC = B // N_CORES          # 2048 batch rows per core
NBT = 4                    # batch tiles per core (512 each)
NBT_W = 512                # batch tile width
NSUB = 4                   # 128-row sub-chunks per batch tile
KD = D // 128              # 32 contraction chunks over D
KL4 = L // 128             # 4 contraction chunks over L


def _build():
    nc = bass.Bass(num_devices=N_CORES)

    # ---- DRAM parameters (inputs) ----
    xT = nc.declare_dram_parameter("xT", [D, BC], F32R, isOutput=False)
    eps = nc.declare_dram_parameter("eps", [BC, L], F32, isOutput=False)
    W1 = nc.declare_dram_parameter("W1", [D, H], F32R, isOutput=False)
    W2mu = nc.declare_dram_parameter("W2mu", [H, L], F32R, isOutput=False)
    W2lv = nc.declare_dram_parameter("W2lv", [H, L], F32R, isOutput=False)
    P = nc.declare_dram_parameter("P", [L, L], F32R, isOutput=False)
    Wd1 = nc.declare_dram_parameter("Wd1", [L, H], F32R, isOutput=False)
    Wd2 = nc.declare_dram_parameter("Wd2", [H, O], F32R, isOutput=False)
    CTm2 = nc.declare_dram_parameter("CTm2", [L, R], F32R, isOutput=False)  # -2*C.T
    cn = nc.declare_dram_parameter("cn", [1, R], F32R, isOutput=False)      # |c|^2
    C = nc.declare_dram_parameter("C", [R, L], F32, isOutput=False)
    b1c = nc.declare_dram_parameter("b1c", [H, 1], F32, isOutput=False)
    bd1c = nc.declare_dram_parameter("bd1c", [H, 1], F32, isOutput=False)
    cmu = nc.declare_dram_parameter("cmu", [1, L], F32R, isOutput=False)    # b2[:L] @ P
    b2lv = nc.declare_dram_parameter("b2lv", [1, L], F32R, isOutput=False)  # b2[L:]
    bd2 = nc.declare_dram_parameter("bd2", [1, O], F32R, isOutput=False)
    vc = nc.declare_dram_parameter("vc", [1, R], F32, isOutput=False)
    onesr = nc.declare_dram_parameter("onesr", [1, 128], F32R, isOutput=False)      # visit_counts

    # ---- DRAM parameters (outputs) ----
    out = nc.declare_dram_parameter("out", [BC, O], F32, isOutput=True)
    kl_o = nc.declare_dram_parameter("kl", [1, 1], F32, isOutput=True)
    ncnt_o = nc.declare_dram_parameter("new_counts", [1, R], F32, isOutput=True)
    ncen_o = nc.declare_dram_parameter("new_centroids", [R, L], F32, isOutput=True)

    with tile.TileContext(nc) as tc:
        with (
            tc.tile_pool(name="consts", bufs=1) as consts,
            tc.tile_pool(name="xin", bufs=16) as xin,
            tc.tile_pool(name="sb", bufs=2) as sb,
            tc.tile_pool(name="mupool", bufs=5) as mupool,
            tc.tile_pool(name="zpool", bufs=5) as zpool,
            tc.tile_pool(name="ztpool", bufs=5) as ztpool,
            tc.tile_pool(name="opool", bufs=4) as opool,
            tc.tile_pool(name="post", bufs=1) as post,
            tc.tile_pool(name="ps", bufs=4, space="PSUM") as ps,
            tc.tile_pool(name="pseg", bufs=1, space="PSUM") as pseg,
            tc.tile_pool(name="pcnt", bufs=1, space="PSUM") as pcnt,
            tc.tile_pool(name="dram", bufs=1, space="DRAM") as dram,
        ):
            # ---- resident constants ----
            w1t = consts.tile([128, KD, H], F32R)
            nc.sync.dma_start(w1t[:], W1.rearrange("(c p) h -> p c h", p=128))
            w2mut = consts.tile([H, L], F32R)
            nc.sync.dma_start(w2mut[:], W2mu[:])
            w2lvt = consts.tile([H, L], F32R)
            nc.sync.dma_start(w2lvt[:], W2lv[:])
            pt = consts.tile([128, KL4, L], F32R)
            nc.sync.dma_start(pt[:], P.rearrange("(c p) l -> p c l", p=128))
            wd1t = consts.tile([128, KL4, H], F32R)
            wd2t = consts.tile([H, O], F32R)
            ctm2t = consts.tile([128, KL4, R], F32R)
            cnt_ = consts.tile([1, R], F32R)
            b1t = consts.tile([H, 1], F32)
            nc.sync.dma_start(b1t[:], b1c[:])
            bd1t = consts.tile([H, 1], F32)
            nc.sync.dma_start(bd1t[:], bd1c[:])
            cmut = consts.tile([1, L], F32R)
            nc.sync.dma_start(cmut[:], cmu[:])
            b2lvt = consts.tile([1, L], F32R)
            nc.sync.dma_start(b2lvt[:], b2lv[:])
            bd2t = consts.tile([1, O], F32R)

            ones_col = consts.tile([128, 1], F32)
            nc.vector.memset(ones_col[:], 1.0)
            ones_row = consts.tile([1, 128], F32R)
            nc.sync.dma_start(ones_row[:], onesr[:])
            ident = consts.tile([128, 128], F32)
            make_identity(nc, ident[:])
            klstrip = consts.tile([128, 48], F32)

            # persistent PSUM accumulators
            p_seg = pseg.tile([R, L], F32)
            p_cnt = pcnt.tile([1, R], F32)

            sub = 0  # global 128-row sub-chunk index (0..15)
            for bt in range(NBT):
                b0 = bt * NBT_W
                # ---- enc1: h1T = relu(W1.T @ xT + b1)  [H, 512] ----
                ph1 = ps.tile([H, NBT_W], F32, tag="ps")
                for kc in range(KD):
                    xt = xin.tile([128, NBT_W], F32R, tag="x")
                    nc.sync.dma_start(
                        xt[:], xT[kc * 128 : (kc + 1) * 128, b0 : b0 + NBT_W]
                    )
                    nc.tensor.matmul(
                        ph1[:], w1t[:, kc, :], xt[:],
                        start=(kc == 0), stop=(kc == KD - 1),
                    )
                h1 = sb.tile([H, NBT_W], F32R, tag="h1")
                nc.scalar.activation(h1[:], ph1[:], AF.Relu, bias=b1t[:])
                if bt == 0:
                    nc.sync.dma_start(wd1t[:], Wd1.rearrange("(c p) h -> p c h", p=128))
                    nc.sync.dma_start(ctm2t[:], CTm2.rearrange("(c p) r -> p c r", p=128))
                    nc.sync.dma_start(cnt_[:], cn[:])
                    nc.sync.dma_start(wd2t[:], Wd2[:])
                    nc.sync.dma_start(bd2t[:], bd2[:])

                # ---- enc2-mu: muT chunks [128, 512] x4 ----
                muT = []
                for c in range(KL4):
                    pm = ps.tile([128, NBT_W], F32, tag="ps")
                    nc.tensor.matmul(
                        pm[:], w2mut[:, c * 128 : (c + 1) * 128], h1[:],
                        start=True, stop=True,
                    )
                    mt = mupool.tile([128, NBT_W], F32R, tag="mu")
                    nc.scalar.activation(mt[:], pm[:], AF.Copy)
                    muT.append(mt)

                zs = []
                for m in range(NSUB):
                    ms = slice(m * 128, (m + 1) * 128)
                    # ---- enc2-logvar (normal layout) + b2lv ----
                    pl = ps.tile([128, L], F32, tag="ps")
                    nc.tensor.matmul(pl[:], h1[:, ms], w2lvt[:], start=True, stop=False)
                    nc.tensor.matmul(pl[:], ones_row[:], b2lvt[:], start=False, stop=True)
                    lv = sb.tile([128, L], F32, tag="lv")
                    nc.scalar.activation(
                        lv[:], pl[:], AF.Copy,
                        accum_out=klstrip[:, sub : sub + 1],
                    )
                    sg = sb.tile([128, L], F32, tag="sg")
                    nc.scalar.activation(sg[:], lv[:], AF.Exp, scale=0.5)
                    sc1 = sb.tile([128, L], F32, tag="scr")
                    nc.scalar.activation(
                        sc1[:], lv[:], AF.Exp,
                        accum_out=klstrip[:, 16 + sub : 17 + sub],
                    )

                    # ---- mu_s (normal layout) + cmu ----
                    pmu = ps.tile([128, L], F32, tag="ps")
                    for kc in range(KL4):
                        nc.tensor.matmul(
                            pmu[:], muT[kc][:, ms], pt[:, kc, :],
                            start=(kc == 0), stop=False,
                        )
                    nc.tensor.matmul(pmu[:], ones_row[:], cmut[:], start=False, stop=True)
                    sc2 = sb.tile([128, L], F32, tag="scr")
                    nc.scalar.activation(
                        sc2[:], pmu[:], AF.Square,
                        accum_out=klstrip[:, 32 + sub : 33 + sub],
                    )

                    # ---- z = mu_s + eps * sigma ----
                    ep = sb.tile([128, L], F32, tag="ep")
                    nc.sync.dma_start(ep[:], eps[b0 + m * 128 : b0 + (m + 1) * 128, :])
                    es = sb.tile([128, L], F32, tag="es")
                    nc.vector.tensor_tensor(es[:], ep[:], sg[:], ALU.mult)
                    z = zpool.tile([128, L], F32, tag="z")
                    nc.vector.tensor_tensor(z[:], pmu[:], es[:], ALU.add)
                    zs.append(z)
                    sub += 1

                # ---- zT: transpose z tiles -> [128(Lc), 512b] x4 ----
                zT = []
                for c in range(KL4):
                    pz = ps.tile([128, NBT_W], F32, tag="ps")
                    for m in range(NSUB):
                        nc.tensor.transpose(
                            pz[:, m * 128 : (m + 1) * 128],
                            zs[m][:, c * 128 : (c + 1) * 128],
                            ident[:],
                        )
                    zt = ztpool.tile([128, NBT_W], F32R, tag="zt")
                    nc.scalar.activation(zt[:], pz[:], AF.Copy)
                    zT.append(zt)

                # ---- dec1: h2T = relu(Wd1.T @ zT + bd1)  [H, 512] ----
                ph2 = ps.tile([H, NBT_W], F32, tag="ps")
                for c in range(KL4):
                    nc.tensor.matmul(
                        ph2[:], wd1t[:, c, :], zT[c][:],
                        start=(c == 0), stop=(c == KL4 - 1),
                    )
                h2 = sb.tile([H, NBT_W], F32R, tag="h2")
                nc.scalar.activation(h2[:], ph2[:], AF.Relu, bias=bd1t[:])

                for m in range(NSUB):
                    ms = slice(m * 128, (m + 1) * 128)
                    gsub = bt * NSUB + m
                    # ---- scores S = -2 z C^T + |c|^2 (f32) ----
                    psc = ps.tile([128, R], F32, tag="ps")
                    for c in range(KL4):
                        nc.tensor.matmul(
                            psc[:], zT[c][:, ms], ctm2t[:, c, :],
                            start=(c == 0), stop=False,
                        )
                    nc.tensor.matmul(psc[:], ones_row[:], cnt_[:], start=False, stop=True)
                    mn = sb.tile([128, 1], F32, tag="mn")
                    nc.vector.tensor_reduce(mn[:], psc[:], mybir.AxisListType.X, ALU.min)
                    A = sb.tile([128, R], F32, tag="A")
                    nc.vector.tensor_scalar(A[:], psc[:], mn[:], None, ALU.is_equal)

                    # ---- segment sums + counts ----
                    nc.tensor.matmul(
                        p_seg[:], A[:], zs[m][:],
                        start=(gsub == 0), stop=(gsub == NBT * NSUB - 1),
                    )
                    nc.tensor.matmul(
                        p_cnt[:], ones_col[:], A[:],
                        start=(gsub == 0), stop=(gsub == NBT * NSUB - 1),
                    )

                    # ---- dec2: out rows [128, O] ----
                    for n in range(O // 512):
                        po = ps.tile([128, 512], F32, tag="ps")
                        nc.tensor.matmul(
                            po[:], h2[:, ms], wd2t[:, n * 512 : (n + 1) * 512],
                            start=True, stop=False,
                        )
                        nc.tensor.matmul(
                            po[:], ones_row[:], bd2t[:, n * 512 : (n + 1) * 512],
                            start=False, stop=True,
                        )
                        ot = opool.tile([128, 512], F32, tag="ot")
                        if n % 2 == 0:
                            nc.scalar.activation(ot[:], po[:], AF.Copy)
                        else:
                            nc.vector.tensor_copy(ot[:], po[:])
                        nc.sync.dma_start(
                            out[b0 + m * 128 : b0 + (m + 1) * 128, n * 512 : (n + 1) * 512],
                            ot[:],
                        )

            # ---- kl partial: sum(klstrip_lv) - sum(mu2) - sum(s2) over all ----
            kl_lv = post.tile([128, 1], F32)
            nc.vector.tensor_reduce(kl_lv[:], klstrip[:, 0:16], mybir.AxisListType.X, ALU.add)
            kl_s2 = post.tile([128, 1], F32)
            nc.vector.tensor_reduce(kl_s2[:], klstrip[:, 16:32], mybir.AxisListType.X, ALU.add)
            kl_m2 = post.tile([128, 1], F32)
            nc.vector.tensor_reduce(kl_m2[:], klstrip[:, 32:48], mybir.AxisListType.X, ALU.add)
            kl_a = post.tile([128, 1], F32)
            nc.vector.tensor_tensor(kl_a[:], kl_lv[:], kl_s2[:], ALU.subtract)
            kl_b = post.tile([128, 1], F32)
            nc.vector.tensor_tensor(kl_b[:], kl_a[:], kl_m2[:], ALU.subtract)
            p_kl = ps.tile([1, 1], F32, tag="ps")
            nc.tensor.matmul(p_kl[:], kl_b[:], ones_col[:], start=True, stop=True)

            # ---- pack AllReduce bounce: rows 0..99 sums, row 100 counts, row 101 kl ----
            ar_in = dram.tile([R + 2, L], F32)
            ar_out = dram.tile([R + 2, L], F32)
            seg_sb = post.tile([R, L], F32)
            nc.scalar.activation(seg_sb[:], p_seg[:], AF.Copy)
            nc.sync.dma_start(ar_in[0:R, :], seg_sb[:])
            cnt_sb = post.tile([1, L], F32)
            nc.vector.memset(cnt_sb[:], 0.0)
            nc.vector.tensor_copy(cnt_sb[:, 0:R], p_cnt[:])
            nc.sync.dma_start(ar_in[R : R + 1, :], cnt_sb[:])
            kl_sb = post.tile([1, L], F32)
            nc.vector.memset(kl_sb[:], 0.0)
            nc.vector.tensor_copy(kl_sb[:, 0:1], p_kl[:])
            nc.sync.dma_start(ar_in[R + 1 : R + 2, :], kl_sb[:])

            nc.gpsimd.collective_compute(
                "AllReduce",
                ALU.add,
                replica_groups=[list(range(N_CORES))],
                ins=[ar_in[:]],
                outs=[ar_out[:]],
            )

            # ---- post-AR (replicated on all cores) ----
            sums_t = post.tile([R, L], F32)
            nc.sync.dma_start(sums_t[:], ar_out[0:R, :])
            cnt_row = post.tile([1, R], F32)
            nc.sync.dma_start(cnt_row[:], ar_out[R : R + 1, 0:R])
            cnt_col = post.tile([R, 1], F32)
            nc.sync.dma_start(
                cnt_col[:], ar_out[R : R + 1, 0:R].rearrange("a (r o) -> (a r) o", o=1)
            )
            klv = post.tile([1, 1], F32)
            nc.sync.dma_start(klv[:], ar_out[R + 1 : R + 2, 0:1])

            # kl_out = -0.005 * (L*B + klsum) / B
            kl_fin = post.tile([1, 1], F32)
            nc.scalar.activation(
                kl_fin[:], klv[:], AF.Copy,
                scale=float(-0.005 / B), bias=float(-0.005 * L),
            )
            nc.sync.dma_start(kl_o[:], kl_fin[:])

            # new_counts = visit_counts + counts
            vct = post.tile([1, R], F32)
            nc.sync.dma_start(vct[:], vc[:])
            ncr = post.tile([1, R], F32)
            nc.vector.tensor_tensor(ncr[:], vct[:], cnt_row[:], ALU.add)
            nc.sync.dma_start(ncnt_o[:], ncr[:])

            # centroid update
            mask = post.tile([R, 1], F32)
            nc.vector.tensor_scalar(mask[:], cnt_col[:], 0.0, None, ALU.is_gt)
            mx = post.tile([R, 1], F32)
            nc.vector.tensor_scalar(mx[:], cnt_col[:], 1.0, None, ALU.max)
            rec = post.tile([R, 1], F32)
            nc.vector.reciprocal(rec[:], mx[:])
            c2a = post.tile([R, 1], F32)
            nc.vector.tensor_tensor(c2a[:], rec[:], mask[:], ALU.mult)
            coef2 = post.tile([R, 1], F32)
            nc.vector.tensor_scalar(coef2[:], c2a[:], 0.01, None, ALU.mult)
            coef1 = post.tile([R, 1], F32)
            nc.vector.tensor_scalar(coef1[:], mask[:], -0.01, 1.0, ALU.mult, ALU.add)

            c_t = post.tile([R, L], F32)
            nc.sync.dma_start(c_t[:], C[:])
            t1 = post.tile([R, L], F32)
            nc.vector.tensor_scalar(t1[:], c_t[:], coef1[:], None, ALU.mult)
            t2 = post.tile([R, L], F32)
            nc.vector.tensor_scalar(t2[:], sums_t[:], coef2[:], None, ALU.mult)
            newc = post.tile([R, L], F32)
            nc.vector.tensor_tensor(newc[:], t1[:], t2[:], ALU.add)
            nc.sync.dma_start(ncen_o[:], newc[:])

    _split_multi_waits(nc)
    return nc


_NC_CACHE = None


def _get_nc():
    global _NC_CACHE
    if _NC_CACHE is None:
        _NC_CACHE = _build()
    return _NC_CACHE


def kernel(x, eps, W1, b1, W2, b2, prior_basis, Wd1, bd1, Wd2, bd2,
           visit_counts, visit_centroids):
    x = np.asarray(x, np.float32)
    eps_np = np.asarray(eps, np.float32)
    W1 = np.asarray(W1, np.float32)
    b1 = np.asarray(b1, np.float32)
    W2 = np.asarray(W2, np.float32)
    b2 = np.asarray(b2, np.float32)
    P = np.asarray(prior_basis, np.float32)
    Wd1 = np.asarray(Wd1, np.float32)
    bd1 = np.asarray(bd1, np.float32)
    Wd2 = np.asarray(Wd2, np.float32)
    bd2 = np.asarray(bd2, np.float32)
    vc = np.asarray(visit_counts, np.float32)
    C = np.asarray(visit_centroids, np.float32)

    cmu = (b2[:L] @ P).reshape(1, L).astype(np.float32)
    shared = {
        "W1": W1,
        "W2mu": np.ascontiguousarray(W2[:, :L]),
        "W2lv": np.ascontiguousarray(W2[:, L:]),
        "P": P,
        "Wd1": Wd1,
        "Wd2": Wd2,
        "CTm2": np.ascontiguousarray(-2.0 * C.T),
        "cn": (C * C).sum(axis=1).reshape(1, R).astype(np.float32),
        "C": C,
        "b1c": b1.reshape(H, 1),
        "bd1c": bd1.reshape(H, 1),
        "cmu": cmu,
        "b2lv": b2[L:].reshape(1, L),
        "bd2": bd2.reshape(1, O),
        "vc": vc.reshape(1, R),
        "onesr": np.ones((1, 128), np.float32),
    }
    in_maps = []
    for c in range(N_CORES):
        sl = slice(c * BC, (c + 1) * BC)
        m = dict(shared)
        m["xT"] = np.ascontiguousarray(x[sl].T)
        m["eps"] = np.ascontiguousarray(eps_np[sl])
        in_maps.append(m)

    nc = _get_nc()
    trace = bool(os.environ.get("BASS_KERNEL_TRACE"))
    if trace:
        from trn_agent_boot.trn_boot import _ntff_profile_via_ctypes
        import antenv.axon_hooks as ah

        if ah.get_axon_ntff_profile_hook() is None:
            ah.set_axon_ntff_profile_hook(
                _ntff_profile_via_ctypes("/opt/axon/libaxon_pjrt.so")
            )
    res = run_bass_kernel_spmd(nc, in_maps, list(range(N_CORES)), trace=trace)
    if trace:
        print(f"HW exec time: {res.exec_time_ns} ns")

    out = np.concatenate([res.results[c]["out"] for c in range(N_CORES)], axis=0)
    kl = np.float32(res.results[0]["kl"][0, 0])
    new_counts = res.results[0]["new_counts"].reshape(R)
    new_centroids = res.results[0]["new_centroids"]
    return out, kl, new_counts, new_centroids
